# revision 11
# baseline (speedup 1.0000x reference)
"""AMP-Net (sparse-view CT unrolled reconstruction) for 8 TRN2 NeuronCores.

Structure
---------
Host (exact, static given theta):
  - sparse Radon forward matrix (bilinear ray sampling, matches
    jax.scipy.ndimage.map_coordinates order=1/constant discretization bit-for-bit
    in structure), Toeplitz ramp-filter matrix, sparse backprojection matrix,
    per-layer pointwise combines.
Device (8 cores, row-slab data parallel, collective-free):
  - the denoiser CNN (1->32->32->32->1, 3x3 convs, ReLU) -- ~92% of the
    model FLOPs -- as TensorEngine matmuls over 3x3 taps with PSUM
    accumulation. Each core computes a 64-row slab of both batch images.
    One compiled graph is reused for all 3 unrolled layers.

Performance notes (measured under the axon-tunneled 8-core setup):
  - Device exec is sub-ms; per-dispatch wall time is dominated by a
    ~86ms axon RPC floor + ~22ms batched output fetch. Hence: build the
    jit(shard_map(bass_exec)) callable ONCE (run_bass_kernel_spmd
    retraces per call, ~400ms extra), bf16 outputs, and device-resident
    non-donated output placeholders uploaded once.
  - 3 dispatches are irreducible while radon/filter/backproj stay on
    host (each layer's CNN input depends on host G of the previous
    noise). Fusing all layers into one dispatch needs exact ray-driven
    radon on device, which reduces to a non-affine 2-D staircase gather
    Y[g_d + h_t] with no access-pattern/selection-matmul form; the
    pixel-driven (splat) radon substitute measures 7.9e-2 end-to-end
    error vs the 2e-2 gate. Backprojection/filter do have exact
    matmul forms (Hankel + 3-tap selection).
"""
import os
import numpy as np
import scipy.sparse as sp

os.environ.setdefault("MYCRO_LOCAL_CACHE", "1")

N = 512
A = 30
D = 729
L = 3
B = 2
P = 2048

NCORES = 8
SLAB = N // NCORES            # 64 output rows per core
HALF = 32                     # sub-slab output rows
GRID_R = 42                   # 1 pad + 40 data + 1 pad rows
GRID_W = 514                  # 1 pad + 512 + 1 pad cols
GRID = GRID_R * GRID_W        # 21588
NBLK = 4                      # B images x 2 halves

_DEV_CACHE = {}


# ----------------------------------------------------------------------------
# Host-side exact operators
# ----------------------------------------------------------------------------

def _filter_matrix():
    n = np.concatenate((np.arange(1, P // 2 + 1, 2), np.arange(P // 2 - 1, 0, -2)))
    f = np.zeros(P)
    f[0] = 0.25
    f[1::2] = -1.0 / (np.pi * n) ** 2
    filt = 2.0 * np.real(np.fft.fft(f))[: P // 2 + 1]
    k = np.fft.irfft(filt, n=P)
    dp = np.arange(D)[:, None]
    d = np.arange(D)[None, :]
    K = k[(d - dp) % P] * (np.pi / (2.0 * A))
    return K.astype(np.float32)


def _geometry_f32(theta):
    c = np.float32((N - 1) / 2.0)
    s = np.arange(D, dtype=np.float32) - np.float32((D - 1) / 2.0)
    t = s
    cs = np.cos(theta.astype(np.float32)).astype(np.float32)
    sn = np.sin(theta.astype(np.float32)).astype(np.float32)
    x = s[None, :, None] * cs[:, None, None] - t[None, None, :] * sn[:, None, None]
    y = s[None, :, None] * sn[:, None, None] + t[None, None, :] * cs[:, None, None]
    return (y + c).astype(np.float32), (x + c).astype(np.float32)


def _radon_sparse(theta):
    rows, cols = _geometry_f32(theta)
    rows = rows.astype(np.float64)
    cols = cols.astype(np.float64)
    i0 = np.floor(rows).astype(np.int64)
    j0 = np.floor(cols).astype(np.int64)
    fr = rows - i0
    fc = cols - j0
    out_idx = np.arange(A)[:, None, None] * D + np.arange(D)[None, :, None]
    out_idx = np.broadcast_to(out_idx, rows.shape)
    data, rr, cc = [], [], []
    for dr in (0, 1):
        for dc in (0, 1):
            ii = i0 + dr
            jj = j0 + dc
            w = (fr if dr else (1.0 - fr)) * (fc if dc else (1.0 - fc))
            m = (ii >= 0) & (ii < N) & (jj >= 0) & (jj < N) & (w != 0)
            data.append(w[m])
            rr.append(out_idx[m])
            cc.append((ii * N + jj)[m])
    R = sp.coo_matrix(
        (np.concatenate(data), (np.concatenate(rr), np.concatenate(cc))),
        shape=(A * D, N * N),
    ).tocsr()
    return R.astype(np.float32)


def _backproj_sparse(theta):
    c = np.float32((N - 1) / 2.0)
    xs = np.arange(N, dtype=np.float32) - c
    cs = np.cos(theta.astype(np.float32)).astype(np.float32)
    sn = np.sin(theta.astype(np.float32)).astype(np.float32)
    sgrid = cs[:, None, None] * xs[None, None, :] + sn[:, None, None] * xs[None, :, None]
    idx = (sgrid + np.float32((D - 1) / 2.0)).astype(np.float32)
    lo = np.floor(idx).astype(np.int64)
    w = (idx - lo).astype(np.float64)
    pix = np.arange(N)[:, None] * N + np.arange(N)[None, :]
    pix = np.broadcast_to(pix[None], (A, N, N))
    aoff = np.arange(A)[:, None, None] * D
    data, rr, cc = [], [], []
    for dtap, wt in ((0, 1.0 - w), (1, w)):
        dd = lo + dtap
        m = (dd >= 0) & (dd < D) & (wt != 0)
        data.append(wt[m])
        rr.append(pix[m])
        cc.append((aoff + dd)[m])
    Bp = sp.coo_matrix(
        (np.concatenate(data), (np.concatenate(rr), np.concatenate(cc))),
        shape=(N * N, A * D),
    ).tocsr()
    return Bp.astype(np.float32)


class _HostOps:
    def __init__(self, theta):
        self.R = _radon_sparse(theta)
        self.K = _filter_matrix()
        self.Bp = _backproj_sparse(theta)

    def radon(self, imgs):
        out = self.R @ imgs.reshape(imgs.shape[0], -1).T
        return out.T.reshape(imgs.shape[0], A, D).astype(np.float32)

    def filt(self, sino):
        return (sino.reshape(-1, D) @ self.K).reshape(sino.shape).astype(np.float32)

    def backproj(self, sino):
        out = self.Bp @ sino.reshape(sino.shape[0], -1).T
        return out.T.reshape(sino.shape[0], N, N).astype(np.float32)


# ----------------------------------------------------------------------------
# Device denoiser
# ----------------------------------------------------------------------------

def _build_device():
    from concourse import bass, bacc, tile
    import concourse.mybir as mybir

    bf16 = mybir.dt.bfloat16
    f32 = mybir.dt.float32
    nc = bacc.Bacc("TRN2", target_bir_lowering=False, debug=False)

    x3_in = nc.dram_tensor("x3_in", [NBLK, 1, GRID], bf16, kind="ExternalInput")
    w1_in = nc.dram_tensor("w1_in", [10, 32], bf16, kind="ExternalInput")
    w2_in = nc.dram_tensor("w2_in", [3, 97, 32], bf16, kind="ExternalInput")
    w3_in = nc.dram_tensor("w3_in", [3, 97, 32], bf16, kind="ExternalInput")
    w4_in = nc.dram_tensor("w4_in", [3, 97, 1], bf16, kind="ExternalInput")
    out_t = nc.dram_tensor("noise_raw", [NBLK, GRID], bf16, kind="ExternalOutput")

    Relu = mybir.ActivationFunctionType.Relu
    Copy = mybir.ActivationFunctionType.Copy

    CH = 512  # psum chunk (max moving free)

    with tile.TileContext(nc) as tc:
        with (
            tc.tile_pool(name="wp", bufs=1) as wp,
            tc.tile_pool(name="big", bufs=1) as big,
            tc.tile_pool(name="x3p", bufs=2) as x3p,
            tc.tile_pool(name="st", bufs=4) as stp,
            tc.tile_pool(name="ps", bufs=6, space="PSUM") as psp,
            tc.tile_pool(name="ps4", bufs=2, space="PSUM") as psp4,
        ):
            w1t = wp.tile([10, 32], bf16, tag="w1")
            w2t = wp.tile([97, 3, 32], bf16, tag="w2")
            w3t = wp.tile([97, 3, 32], bf16, tag="w3")
            w4t = wp.tile([97, 3, 1], bf16, tag="w4")
            nc.sync.dma_start(w1t[:], w1_in[:])
            # dram layout [3, K, M] -> sbuf [K, 3, M]
            for dy in range(3):
                nc.sync.dma_start(w2t[:, dy, :], w2_in[dy])
                nc.sync.dma_start(w3t[:, dy, :], w3_in[dy])
                nc.sync.dma_start(w4t[:, dy, :], w4_in[dy])

            for blk in range(NBLK):
                # X3 layout: p0 = ones (bias row); p1..p9 = x shifted by
                # off = dy*514 + dx for (dy, dx) in row-major (-1,0,1)^2, so
                # conv1 is a single K=10 matmul per chunk. Shifts built via
                # DRAM->SBUF DMAs with free offsets; compute-engine ops stay
                # partition-0 aligned. Flat-wrap edges land in zero pad cols.
                X3 = x3p.tile([10, GRID], bf16, tag="x3")
                nc.vector.memset(X3[:], 0.0)
                nc.vector.memset(X3[0:1, :], 1.0)
                p = 1
                for dy in (-1, 0, 1):
                    for dx in (-1, 0, 1):
                        off = dy * GRID_W + dx
                        if off >= 0:
                            nc.sync.dma_start(X3[p:p + 1, 0:GRID - off],
                                              x3_in[blk][0:1, off:GRID])
                        else:
                            nc.sync.dma_start(X3[p:p + 1, -off:GRID],
                                              x3_in[blk][0:1, 0:GRID + off])
                        p += 1
                Ma = big.tile([97, GRID], bf16, tag="A")
                Mb = big.tile([97, GRID], bf16, tag="B")
                for M97 in (Ma, Mb):
                    nc.vector.memset(M97[96:97, :], 1.0)

                def conv(dst97, srcAP, wt, kparts, f_lo, f_hi, relu=True,
                         out_dram_off=None):
                    """conv over flat grid range [f_lo, f_hi) in chunks.
                    srcAP(dy, off, size) -> moving AP [kparts, size].
                    dst97: write bf16 relu outputs into dst97[32:64].
                    If out_dram_off is not None: conv4 mode, write f32 chunks
                    to out_t[blk, off:off+size]."""
                    if out_dram_off is not None:
                        # conv4: batch 4 psum chunks into one stage tile and
                        # one output DMA (42 -> 11 DMAs per blk)
                        g = f_lo
                        while g < f_hi:
                            gsize = min(4 * CH, f_hi - g)
                            st = stp.tile([1, 4 * CH], bf16, tag="st")
                            q = 0
                            while q < gsize:
                                size = min(CH, gsize - q)
                                pt = psp4.tile([1, CH], f32, tag="pt4")
                                for idy, dy in enumerate((-1, 0, 1)):
                                    nc.tensor.matmul(
                                        pt[:1, :size], wt[:kparts, idy, :1],
                                        srcAP(dy, g + q, size),
                                        start=(idy == 0), stop=(idy == 2),
                                    )
                                nc.scalar.activation(st[:, q:q + size],
                                                     pt[:1, :size], Copy)
                                q += size
                            nc.sync.dma_start(out_t[blk:blk + 1, g:g + gsize],
                                              st[0:1, :gsize])
                            g += gsize
                        return
                    f = f_lo
                    while f < f_hi:
                        size = min(CH, f_hi - f)
                        pt = psp.tile([32, CH], f32, tag="pt")
                        for idy, dy in enumerate((-1, 0, 1)):
                            nc.tensor.matmul(
                                pt[:, :size],
                                wt[:kparts, idy, :32],
                                srcAP(dy, f, size),
                                start=(idy == 0),
                                stop=(idy == 2),
                            )
                        nc.scalar.activation(
                            dst97[32:64, f:f + size], pt[:, :size],
                            Relu if relu else Copy)
                        f += size

                def fixup_and_shift(M97):
                    # re-zero pad rows (0, 41) and pad cols (0, 513) of center
                    nc.vector.memset(M97[32:64, 0:GRID_W], 0.0)
                    nc.vector.memset(M97[32:64, GRID - GRID_W:GRID], 0.0)
                    colsAP = M97[32:64].rearrange("p (r w) -> p r w", r=GRID_R)
                    nc.vector.memset(colsAP[:, :, 0:1], 0.0)
                    nc.vector.memset(colsAP[:, :, GRID_W - 1:GRID_W], 0.0)
                    # dx=-1 block: dst[f] = center[f-1]; dx=+1: dst[f] = center[f+1]
                    nc.sync.dma_start(M97[0:32, 1:GRID], M97[32:64, 0:GRID - 1])
                    nc.sync.dma_start(M97[64:96, 0:GRID - 1], M97[32:64, 1:GRID])
                    nc.vector.memset(M97[0:32, 0:1], 0.0)
                    nc.vector.memset(M97[64:96, GRID - 1:GRID], 0.0)

                # conv1: x3 -> Ma  (valid grid rows 1..40); single K=10 matmul
                f = GRID_W
                while f < 41 * GRID_W:
                    size = min(CH, 41 * GRID_W - f)
                    pt = psp.tile([32, CH], f32, tag="pt")
                    nc.tensor.matmul(pt[:, :size], w1t[:],
                                     X3[:, f:f + size], start=True, stop=True)
                    nc.scalar.activation(Ma[32:64, f:f + size], pt[:, :size],
                                         Relu)
                    f += size
                fixup_and_shift(Ma)
                # conv2: Ma -> Mb
                conv(Mb,
                     lambda dy, f, s: Ma[:, f + dy * GRID_W: f + dy * GRID_W + s],
                     w2t, 97, GRID_W, 41 * GRID_W)
                fixup_and_shift(Mb)
                # conv3: Mb -> Ma  (Ma's conv1 contents fully consumed)
                conv(Ma,
                     lambda dy, f, s: Mb[:, f + dy * GRID_W: f + dy * GRID_W + s],
                     w3t, 97, GRID_W, 41 * GRID_W)
                fixup_and_shift(Ma)
                # conv4: Ma -> out (full data window; host crops per-core)
                conv(None,
                     lambda dy, f, s: Ma[:, f + dy * GRID_W: f + dy * GRID_W + s],
                     w4t, 97, GRID_W, 41 * GRID_W, relu=False,
                     out_dram_off=0)

    nc.compile()
    return nc, (x3_in, w1_in, w2_in, w3_in, w4_in, out_t)


def _get_device():
    if "nc" not in _DEV_CACHE:
        _DEV_CACHE["nc"] = _build_device()
    return _DEV_CACHE["nc"]


def _build_cached_call(nc, n_cores, replicate_out=False):
    """One-time jit(shard_map(bass_exec)) construction; warm calls skip
    retracing/relowering (run_bass_kernel_spmd rebuilds the jit closure
    every call, paying ~400ms of tracing + compile-cache lookups)."""
    import jax
    from jax.sharding import Mesh, PartitionSpec
    from jax.experimental.shard_map import shard_map
    from concourse import bass2jax
    import concourse.mybir as mybir

    bass2jax.install_neuronx_cc_hook()
    partition_name = nc.partition_id_tensor.name if nc.partition_id_tensor else None

    in_names, out_names, out_avals, zero_outs = [], [], [], []
    for alloc in nc.m.functions[0].allocations:
        if not isinstance(alloc, mybir.MemoryLocationSet):
            continue
        name = alloc.memorylocations[0].name
        if alloc.kind == "ExternalInput":
            if name != partition_name:
                in_names.append(name)
        elif alloc.kind == "ExternalOutput":
            shape = tuple(alloc.tensor_shape)
            dtype = mybir.dt.np(alloc.dtype)
            out_avals.append(jax.core.ShapedArray(shape, dtype))
            zero_outs.append(np.zeros(shape, dtype))
            out_names.append(name)
    n_params = len(in_names)
    n_outs = len(out_avals)
    all_in_names = list(in_names) + list(out_names)
    if partition_name is not None:
        all_in_names.append(partition_name)

    def _body(*args):
        operands = list(args)
        if partition_name is not None:
            operands.append(bass2jax.partition_id_tensor())
        outs = bass2jax._bass_exec_p.bind(
            *operands,
            out_avals=tuple(out_avals),
            in_names=tuple(all_in_names),
            out_names=tuple(out_names),
            lowering_input_output_aliases=(),
            sim_require_finite=True,
            sim_require_nnan=True,
            nc=nc,
        )
        return tuple(outs)

    devices = jax.devices()[:n_cores]
    mesh = Mesh(np.asarray(devices), ("core",))
    in_specs = (PartitionSpec("core"),) * (n_params + n_outs)
    out_specs = (PartitionSpec("core"),) * n_outs
    smapped = shard_map(_body, mesh=mesh, in_specs=in_specs,
                        out_specs=out_specs, check_rep=False)
    if replicate_out:
        # All-gather outputs on-device so the host fetch is one RPC
        # instead of one per shard (~26ms -> ~3ms for 1.4MB).
        from jax.sharding import NamedSharding
        rep = NamedSharding(mesh, PartitionSpec())

        def fn(*args):
            return tuple(jax.lax.with_sharding_constraint(o, rep)
                         for o in smapped(*args))
    else:
        fn = smapped
    sharded = jax.jit(fn, keep_unused=True)
    # Output placeholder buffers: without donation the bass custom call
    # allocates fresh outputs, so these device-resident zeros are never
    # mutated and can be uploaded once and reused across calls. Unwritten
    # output cells are then uninitialized, which is fine: the host unpack
    # only reads grid rows [1, 41), all written by conv4.
    from jax.sharding import NamedSharding
    sh = NamedSharding(mesh, PartitionSpec("core"))
    zeros_dev = [jax.device_put(
        np.zeros((n_cores * z.shape[0], *z.shape[1:]), z.dtype), sh)
        for z in zero_outs]

    def call(in_maps):
        per_core = [[np.asarray(m[name]) for name in in_names] for m in in_maps]
        concat_in = [np.concatenate([per_core[c][i] for c in range(n_cores)],
                                    axis=0) for i in range(n_params)]
        out_arrs = sharded(*concat_in, *zeros_dev)
        return [
            {name: np.asarray(out_arrs[i]).reshape(n_cores, *out_avals[i].shape)[c]
             for i, name in enumerate(out_names)}
            for c in range(n_cores)
        ]

    call.sharded = sharded
    call.in_names = in_names
    call.out_names = out_names
    call.out_avals = out_avals
    call.zeros_dev = zeros_dev
    call.mesh = mesh
    return call


def _get_call():
    if "call" not in _DEV_CACHE:
        nc, _ = _get_device()
        _DEV_CACHE["call"] = _build_cached_call(nc, NCORES)
    return _DEV_CACHE["call"]


def _pack_weights(w1, b1, w2, b2, w3, b3, w4):
    """Per-layer stationaries, bf16. Returns dict of [3, K, M] arrays."""
    import ml_dtypes
    bf = ml_dtypes.bfloat16

    W1 = np.zeros((10, 32), np.float32)
    W2 = np.zeros((3, 97, 32), np.float32)
    W3 = np.zeros((3, 97, 32), np.float32)
    W4 = np.zeros((3, 97, 1), np.float32)
    for idy in range(3):
        for k in range(3):  # dx index
            W1[1 + 3 * idy + k, :] = w1[:, 0, idy, k]
            W2[idy, 32 * k:32 * k + 32, :] = w2[:, :, idy, k].T
            W3[idy, 32 * k:32 * k + 32, :] = w3[:, :, idy, k].T
            W4[idy, 32 * k:32 * k + 32, 0] = w4[0, :, idy, k]
    W1[0, :] = b1
    W2[1, 96, :] = b2
    W3[1, 96, :] = b3
    return {
        "w1_in": W1.astype(bf),
        "w2_in": W2.astype(bf),
        "w3_in": W3.astype(bf),
        "w4_in": W4.astype(bf),
    }


def _grid_placement(r0):
    """Place x rows so a global image edge coincides with a grid pad row
    (grid pad rows are stage-wise re-zeroed on device = SAME-conv padding).
    Returns (start_x, rows, g_data0, g_out0)."""
    halo_top = min(4, r0)
    start_x = r0 - halo_top
    end_x = min(N, r0 + HALF + 4)
    rows = end_x - start_x
    if end_x == N:
        g_data0 = 41 - rows   # image bottom edge at grid row 41 (pad)
    else:
        g_data0 = 1           # top halo (or image top edge) at grid row 1
    g_out0 = g_data0 + halo_top
    return start_x, rows, g_data0, g_out0


def _pack_x3(X):
    """X [B, N, N] f32 -> per-core x3 arrays [NBLK, 4, GRID] bf16."""
    import ml_dtypes
    bf = ml_dtypes.bfloat16
    per_core = []
    for core in range(NCORES):
        r0c = core * SLAB
        blks = np.zeros((NBLK, 1, GRID), np.float32)
        bi = 0
        for im in range(B):
            for h in range(2):
                r0 = r0c + h * HALF
                start_x, rows, g_data0, _ = _grid_placement(r0)
                xg = np.zeros((GRID_R, GRID_W), np.float32)
                xg[g_data0:g_data0 + rows, 1:513] = X[im, start_x:start_x + rows, :]
                blks[bi, 0, :] = xg.reshape(-1)
                bi += 1
        per_core.append(blks.astype(bf))
    return per_core


def _device_denoiser(X, wpack):
    """X [B, N, N] f32 -> noise [B, N, N] f32 via the 8-core device kernel."""
    call = _get_call()
    x3s = _pack_x3(X)
    in_maps = []
    for core in range(NCORES):
        m = {"x3_in": x3s[core]}
        m.update(wpack)
        in_maps.append(m)
    results = call(in_maps)
    noise = np.zeros((B, N, N), np.float32)
    for core in range(NCORES):
        raw = results[core]["noise_raw"]  # [NBLK, GRID]
        bi = 0
        for im in range(B):
            for h in range(2):
                r0 = core * SLAB + h * HALF
                _, _, _, g_out0 = _grid_placement(r0)
                g = raw[bi].reshape(GRID_R, GRID_W)
                noise[im, r0:r0 + HALF, :] = g[g_out0:g_out0 + HALF, 1:513]
                bi += 1
    return noise


# ----------------------------------------------------------------------------
# Entry point
# ----------------------------------------------------------------------------

def kernel(cond, x0, sinogram, theta, theta_label, w1, b1, w2, b2, w3, b3, w4,
           steps, **_):
    x0 = np.asarray(x0, np.float32)
    sinogram = np.asarray(sinogram, np.float32)
    theta = np.asarray(theta, np.float32)
    w1 = np.asarray(w1, np.float32); b1 = np.asarray(b1, np.float32)
    w2 = np.asarray(w2, np.float32); b2 = np.asarray(b2, np.float32)
    w3 = np.asarray(w3, np.float32); b3 = np.asarray(b3, np.float32)
    w4 = np.asarray(w4, np.float32)
    steps = np.asarray(steps, np.float32)

    key = tuple(np.round(theta, 7).tolist())
    if _DEV_CACHE.get("ops_key") != key:
        _DEV_CACHE["ops"] = _HostOps(theta)
        _DEV_CACHE["ops_key"] = key
    ops = _DEV_CACHE["ops"]

    X = x0[:, 0]
    sino = sinogram[:, 0]
    for n in range(L):
        sino_pred = ops.radon(X)
        filtered = ops.filt(sino_pred)
        outputs = ops.backproj(sino - filtered)
        z = X + steps[n] * outputs
        wpack = _pack_weights(w1[n], b1[n], w2[n], b2[n], w3[n], b3[n], w4[n])
        noise = _device_denoiser(X, wpack)
        fn = ops.filt(ops.radon(noise))
        outputs_noise = ops.backproj(fn)
        X = (z + noise - steps[n] * outputs_noise).astype(np.float32)
    Xo = X[:, None]
    return (Xo, Xo, Xo)

